# revision 101
# baseline (speedup 1.0000x reference)
"""CHOWDER-style MIL kernel for Trainium2 (Bass/Tile), 8-core data-parallel.

Per core (4 slides):
  scores = sigmoid(x @ w1.T + b1) @ w2.T          x: (10000, 768) per slide
  extreme = top100(scores) ++ bottom100(scores)   per slide, sorted
  y = mlp(extreme + sb2)                          200 -> 128 -> 64 -> 1

Host preprocessing: features cast to fp8 e4m3 in a blocked per-macro layout
(one contiguous KC*nq run per partition per macro), w1 prescaled x16 into
fp8 (undone via the activation scale), sb2 folded into the slide-MLP
layer-1 bias, the bottom-100 sign flip folded into the m1 columns, all
small fp16 weights packed into one blob, biases uploaded as 4 row
descriptors and transposed on-chip.

Streaming: ~1536-column tile-aligned macros alternate between the two
HWDGE rings; slide 0 ramps with finer (512/1024) leading macros so the PE
starts ~3us after the rings open.  Slide b+1's macro triggers are emitted
a few tiles INTO slide b, when their buffer WAR (18-deep xmac pool) is
already resolved, so neither ring queue ever blocks on compute.  Layer-1
is 3 fp8 DoubleRow (double-pumped) accumulating matmuls per 512-tile;
layer-2 (emitted one tile late so the PE never waits on the sigmoid)
lands all 4 slides' scores in one NEG-padded PSUM bank.

Scores land in two ALTERNATING per-slide PSUM tiles (re-padded after each
slide's reads) so slide b+1's layer-2 writes never WAR-wait on slide b's
topk reads, which sit behind a ring round-trip.

Top-k (all fp16): per direction one max8 pass over the PSUM scores -> [128,
8+8] candidates, ONE reshape DMA -> [16, 8, 16], top-16 per 8-partition
group (observed worst case 15), gathered to a [4, 256] per-pair array in
[top, bot] row pairs, then an exact 13-round max8 + match_replace pass
yields the sorted top-104 and each pair's extreme vector is ONE linear DMA.
Mid-stream slides route the tourney-dependent gathers through the gpsimd
SWDGE queue (disjoint semaphore pool from the macro rings); the last slide
uses the then-idle HWDGE rings.  Pair {0,1} finishes under the streaming
of slides {2,3}, so only the last pair's reduction (~20us) is exposed.

Measured: ~125-136us on hardware (vs 211us fp16 baseline), HBM read 31MB
per core at a ~410 GB/s dual-ring streaming rate.  Data-dependent DMA
triggers must NEVER sit on a ring ahead of macro triggers (ring head
blocks); moving work "earlier" in program order on the sync ring twice
REGRESSED for exactly that reason.
"""

import numpy as np

# Problem constants (hardcoded per harness contract)
B = 32
N = 10000
D = 768
META = 3
NCORES = 8
BPC = B // NCORES          # slides per core
NT = 512                   # n-tile size (PSUM bank = 512 fp32)
KC = D // 128              # 6 contraction chunks
# tile-aligned DMA macros: ~2.9us delivery granularity matches the PE's
# 3-tile pace; slide 0 ramps with finer leading macros so compute starts
# as soon as the first 512 columns land
MACROS0 = [512, 1024, 1536, 1536, 1536, 1536, 1536, 784]
MACROSR = [1536, 1536, 1536, 1536, 1536, 1536, 784]
MACMAX = 1536
NTOP = 100
NROUNDS = 13               # 13*8 = 104 >= 100
SCOL = 80                  # score columns per slide (ceil(10000/128))
NEG = -1e30
NEG16 = -60000.0           # fp16-representable sentinel for the topk chain
WSCALE = 16.0              # w1 prescale so fp8 e4m3 stays in normal range

_PROG = None
LAST_RESULT = None         # BassKernelResults of the most recent run (for test.py)


def _build():
    import concourse.bacc as bacc
    import concourse.mybir as mybir
    from concourse.tile import TileContext
    from concourse.masks import make_identity
    from contextlib import ExitStack

    f8 = mybir.dt.float8e4
    f16 = mybir.dt.float16
    f32 = mybir.dt.float32
    SIG = mybir.ActivationFunctionType.Sigmoid

    nc = bacc.Bacc("TRN2", target_bir_lowering=False, debug=False,
                   enable_asserts=False)

    # per-partition layout: [macro0: KC*nq0 | macro1: KC*nq1 | ...] so every
    # macro is one contiguous run per partition regardless of its width
    xt = nc.dram_tensor("xt", [BPC, 128, KC * N], f8, kind="ExternalInput")
    # pre-blocked on host: w1t[p, k*128+h] = w1[d=128k+p, h], one contiguous
    # 768B descriptor per partition
    w1t = nc.dram_tensor("w1t", [128, KC * 128], f8, kind="ExternalInput")
    # all small fp16 weights packed into one blob (cols: m1a 0:128,
    # m2t 128:192, m1b 192:320, m3t 320, w2t 321) and the fp32 biases into
    # another (sb1, mb1, mb2, mb3) - 3 const DMAs total so the ring
    # semaphore rotation is not polluted at startup
    wblob = nc.dram_tensor("wblob", [128, 322], f16, kind="ExternalInput")
    # biases uploaded as 4 row-descriptors and transposed on-chip (a [128,1]
    # column upload costs 128 tiny descriptors on the ring)
    brows = nc.dram_tensor("brows", [4, 128], f32, kind="ExternalInput")
    y = nc.dram_tensor("y", [1, BPC], f32, kind="ExternalOutput")

    with TileContext(nc) as tc, ExitStack() as ctx:
        const = ctx.enter_context(tc.tile_pool(name="const", bufs=1))
        xpool = ctx.enter_context(tc.tile_pool(name="xp", bufs=18))
        hpool = ctx.enter_context(tc.tile_pool(name="hp", bufs=3))
        tkpool = ctx.enter_context(tc.tile_pool(name="tk", bufs=1))
        negpool = ctx.enter_context(tc.tile_pool(name="ng", bufs=2))
        candpool = ctx.enter_context(tc.tile_pool(name="cd", bufs=4))
        ph_pool = ctx.enter_context(tc.tile_pool(name="ph", bufs=3, space="PSUM"))
        psc_pool = ctx.enter_context(tc.tile_pool(name="psc", bufs=2, space="PSUM"))
        pm_pool = ctx.enter_context(tc.tile_pool(name="pm", bufs=1, space="PSUM"))

        # ---- constants: w1 on sync (first L1 needs it); fp16 weight blob
        # rides the idle SWDGE queue (needed only at kernel end); biases
        # arrive as 4 row-descriptors on scalar and are transposed on-chip
        w1t_sb = const.tile([128, KC, 128], f8, tag="w1t")
        nc.sync.dma_start(out=w1t_sb, in_=w1t[:, :].rearrange("p (k h) -> p k h", k=KC))
        wblob_sb = const.tile([128, 322], f16, tag="wblob")
        nc.gpsimd.dma_start(out=wblob_sb, in_=wblob[:, :])
        brows_sb = const.tile([4, 128], f32, tag="brows")
        nc.scalar.dma_start(out=brows_sb, in_=brows[:, :])
        ident = const.tile([4, 4], f16, tag="ident")
        make_identity(nc, ident)
        ident32 = const.tile([4, 4], f32, tag="ident32")
        make_identity(nc, ident32)
        pbias = pm_pool.tile([128, 4], f32, tag="pbias")
        nc.tensor.transpose(pbias, brows_sb, ident32)
        bias_t = const.tile([128, 4], f32, tag="bias_t")
        nc.scalar.copy(bias_t, pbias)
        m1a_sb = wblob_sb[:, 0:128]
        m2t_sb = wblob_sb[0:128, 128:192]
        m1b_sb = wblob_sb[0:72, 192:320]
        m3t_sb = wblob_sb[0:64, 320:321]
        w2t_sb = wblob_sb[:, 321:322]
        sb1_sb = bias_t[:, 0:1]
        mb1_sb = bias_t[:, 1:2]
        mb2_sb = bias_t[0:64, 2:3]
        mb3_sb = bias_t[0:1, 3:4]

        # tournament: [128, 16] (top cols 0:8 ++ bot cols 8:16) -> one DMA ->
        # [16, 8, 16] (group, src-partition, col) -> top-16 per direction.
        # The whole candidate chain runs in fp16 (2x DVE throughput).
        def tourney_reduce(r1dir, name):
            r2 = candpool.tile([16, 16], f16, tag="r2", name=f"r2{name}")
            nc.vector.max(out=r2[:, 0:8], in_=r1dir)
            nc.vector.match_replace(out=r1dir, in_to_replace=r2[:, 0:8],
                                    in_values=r1dir, imm_value=NEG16)
            nc.vector.max(out=r2[:, 8:16], in_=r1dir)
            return r2

        # exact sorted top-104 of a [4, KEEP*16] candidate array
        def stage2(s2, tag):
            t104 = tkpool.tile([4, NROUNDS * 8], f16, tag=tag)
            for r in range(NROUNDS):
                nc.vector.max(out=t104[:, r * 8 : (r + 1) * 8], in_=s2)
                if r < NROUNDS - 1:
                    nc.vector.match_replace(
                        out=s2, in_to_replace=t104[:, r * 8 : (r + 1) * 8],
                        in_values=s2, imm_value=NEG16)
            return t104

        KEEP = 16   # candidates kept per 8-partition group (worst case seen: 15)
        sbatch = [tkpool.tile([4, KEEP * 16], f16, tag=f"s2_{i}", name=f"s2_{i}")
                  for i in range(2)]
        t104s = [None, None]
        dmacnt = 0

        # two alternating per-slide PSUM score tiles, NEG-padded upfront; the
        # layer-2 matmuls land scores there directly (no scalar copies), and
        # slide b+1's L2 writes never WAR-wait on slide b's topk reads
        pscs = [psc_pool.tile([128, SCOL], f32, tag="psc", name=f"psc{i}")
                for i in range(2)]
        nc.vector.memset(pscs[0], NEG)
        nc.vector.memset(pscs[1], NEG)

        # layer-2 for tile (b, col) — emitted one tile late so the PE never
        # stalls on the scalar sigmoid of the same tile
        def emit_l2(p):
            h, b, col, nj_full, rem, nt = p
            psc = pscs[b % 2]
            for j in range(nj_full):
                nc.tensor.matmul(psc[:, col + j : col + j + 1],
                                 lhsT=h[:, j * 128 : (j + 1) * 128],
                                 rhs=w2t_sb, start=True, stop=True,
                                 skip_group_check=True)
            if rem:
                nc.tensor.matmul(psc[:rem, col + nj_full : col + nj_full + 1],
                                 lhsT=h[:, nj_full * 128 : nt],
                                 rhs=w2t_sb, start=True, stop=True,
                                 skip_group_check=True)

        # macro triggers for slide b+1 are emitted a few tiles INTO slide b:
        # the buffer WAR is already resolved then, so the trigger issues
        # instantly and never blocks the scalar queue's ACTIVATE stream,
        # while the ring stays a full slide ahead of compute
        xmacs = {}

        def table(b):
            return MACROS0 if b == 0 else MACROSR

        def trigger_macro(b, m):
            nonlocal dmacnt
            tab = table(b)
            nq = tab[m]
            off = KC * sum(tab[:m])
            xmac = xpool.tile([128, KC, MACMAX], f8, tag="xmac")
            eng = nc.sync if dmacnt % 2 == 0 else nc.scalar
            dmacnt += 1
            eng.dma_start(
                out=xmac[:, :, :nq],
                in_=xt[b, :, off : off + KC * nq].rearrange("p (k n) -> p k n", k=KC))
            xmacs[(b, m)] = xmac

        pend = None
        for m in range(len(MACROS0)):
            trigger_macro(0, m)
        # ---- streaming phase ----
        for b in range(BPC):
            npos = 0   # position within slide; score col = npos // 128
            tix = 0    # tile index within slide (20 per slide)
            for m, nq in enumerate(table(b)):
                xmac = xmacs.pop((b, m))
                for t0 in range(0, nq, NT):
                    nt = min(NT, nq - t0)
                    col = npos // 128
                    ph = ph_pool.tile([128, NT], f32, tag="ph")
                    for k in range(KC // 2):
                        nc.tensor.matmul(
                            ph[:, :nt],
                            lhsT=w1t_sb[:, 2 * k : 2 * k + 2, :],
                            rhs=xmac[:, 2 * k : 2 * k + 2, t0 : t0 + nt],
                            start=(k == 0), stop=(k == KC // 2 - 1),
                            perf_mode=mybir.MatmulPerfMode.DoubleRow)
                    h = hpool.tile([128, NT], f16, tag="h")
                    nc.scalar.activation(h[:, :nt], ph[:, :nt], SIG,
                                         bias=sb1_sb, scale=1.0 / WSCALE)
                    if pend is not None:
                        emit_l2(pend)
                    pend = (h, b, col, nt // 128, nt - (nt // 128) * 128, nt)
                    npos += nt
                    tix += 1
                    if b + 1 < BPC and 1 <= tix < 1 + len(MACROSR):
                        trigger_macro(b + 1, tix - 1)
            # slide's last tile must land before its top-k reads the scores
            emit_l2(pend)
            pend = None

            # ---- per-slide candidate extraction.  The merged candidate DMA
            # (r1) rides the sync ring: its data dep (the max8s) resolves
            # ~1us after the slide ends, well before the ring works through
            # the next slide's already-queued macros.  The sbatch gathers
            # depend on the slow ring->DVE->ring tourney chain, so mid-stream
            # slides push them through the gpsimd SWDGE queue; the last slide
            # uses the (by then idle) HWDGE rings. ----
            # sbatch gathers depend on the slow ring->DVE->ring tourney
            # chain, so mid-stream slides push them through the gpsimd SWDGE
            # queue (its semaphores are disjoint from the macro rings'); the
            # last slide uses the then-idle HWDGE rings
            if b < BPC - 1:
                eng_t = eng_b = nc.gpsimd
            else:
                eng_t, eng_b = nc.sync, nc.scalar
            psc = pscs[b % 2]
            c1 = candpool.tile([128, 16], f16, tag="c1", name=f"c1{b}")
            nc.vector.max(out=c1[:, 0:8], in_=psc)
            last_rem = N - (N // 128) * 128           # 16 valid rows in col 78
            neg = negpool.tile([128, SCOL], f16, tag="neg")
            nc.vector.memset(neg, NEG16)
            nc.vector.tensor_scalar_mul(neg[:, 0 : N // 128],
                                        psc[:, 0 : N // 128], -1.0)
            if last_rem:
                nc.vector.tensor_scalar_mul(
                    neg[:last_rem, N // 128 : N // 128 + 1],
                    psc[:last_rem, N // 128 : N // 128 + 1], -1.0)
            nc.vector.max(out=c1[:, 8:16], in_=neg)
            if b + 2 < BPC:
                # re-pad for slide b+2 right after this slide's reads; its
                # first L2 write is ~2 slides away, far past this memset
                nc.vector.memset(psc, NEG)
            r1 = candpool.tile([16, 8, 16], f16, tag="r1", name=f"r1{b}")
            if b < BPC - 1:
                # sync ring: fast, and by the time the ring reaches this the
                # max8s are long done, so the stream never stalls (SWDGE here
                # measured worse: its slow roundtrip delays the DVE chain)
                nc.sync.dma_start(out=r1, in_=c1)
            else:
                # split directions across the two idle rings: the top half
                # fires ~0.5us before the bottom max8 lands
                nc.sync.dma_start(out=r1[:, :, 0:8], in_=c1[:, 0:8])
                nc.scalar.dma_start(out=r1[:, :, 8:16], in_=c1[:, 8:16])
            r2_top = tourney_reduce(r1[:, :, 0:8], f"t{b}")
            r2_bot = tourney_reduce(r1[:, :, 8:16], f"b{b}")

            half, q = divmod(b, 2)
            # row layout [s0_top, s0_bot, s1_top, s1_bot] so each slide's ext
            # assembly later is one linear DMA
            eng_t.dma_start(out=sbatch[half][2 * q : 2 * q + 1, :],
                            in_=r2_top[:, :KEEP])
            eng_b.dma_start(out=sbatch[half][2 * q + 1 : 2 * q + 2, :],
                            in_=r2_bot[:, :KEEP])
            if q == 1:
                # pair complete -> exact reduction (hidden under later
                # streaming for the first pair)
                t104s[half] = stage2(sbatch[half], f"t104_{half}")

        # extreme vector [4, 200] = top100 ++ max8(-s)100 per slide; the
        # bottom half's sign flip is folded into the m1 weights on host.
        # Each pair's rows are one linear DMA (rings run in parallel).
        ext = tkpool.tile([4, 200], f16, tag="ext")
        nc.sync.dma_start(out=ext[0:2, :], in_=t104s[0][:, 0:NTOP])
        nc.scalar.dma_start(out=ext[2:4, :], in_=t104s[1][:, 0:NTOP])

        # ---- slide MLP (sb2 folded into mb1 on host) ----
        pt1 = pm_pool.tile([128, 4], f16, tag="pmlp")
        nc.tensor.transpose(pt1, ext[:, 0:128], ident)
        et1 = tkpool.tile([128, 4], f16, tag="et1")
        nc.scalar.copy(et1, pt1)
        pt2 = pm_pool.tile([72, 4], f16, tag="pmlp")
        nc.tensor.transpose(pt2, ext[:, 128:200], ident)
        et2 = tkpool.tile([72, 4], f16, tag="et2")
        nc.scalar.copy(et2, pt2)

        ph1 = pm_pool.tile([128, 4], f32, tag="pmlp")
        nc.tensor.matmul(ph1, lhsT=m1a_sb, rhs=et1, start=True, stop=False)
        nc.tensor.matmul(ph1, lhsT=m1b_sb, rhs=et2, start=False, stop=True)
        h1 = tkpool.tile([128, 4], f16, tag="h1")
        nc.scalar.activation(h1, ph1, SIG, bias=mb1_sb)

        ph2 = pm_pool.tile([64, 4], f32, tag="pmlp")
        nc.tensor.matmul(ph2, lhsT=m2t_sb, rhs=h1, start=True, stop=True)
        h2 = tkpool.tile([64, 4], f16, tag="h2")
        nc.scalar.activation(h2, ph2, SIG, bias=mb2_sb)

        py = pm_pool.tile([1, 4], f32, tag="pmlp")
        nc.tensor.matmul(py, lhsT=m3t_sb, rhs=h2, start=True, stop=True)
        y_sb = tkpool.tile([1, 4], f32, tag="ysb")
        nc.vector.tensor_add(y_sb, py, mb3_sb.to_broadcast([1, 4]))
        nc.sync.dma_start(out=y[:, :], in_=y_sb)

    nc.compile()
    return nc


def _get_prog():
    global _PROG
    if _PROG is None:
        _PROG = _build()
    return _PROG


def kernel(**inputs):
    global LAST_RESULT
    from concourse.bass_utils import run_bass_kernel_spmd

    nc = _get_prog()

    f = np.asarray(inputs["features"], dtype=np.float32)
    sw1 = np.asarray(inputs["sw1"], dtype=np.float32)
    sb1 = np.asarray(inputs["sb1"], dtype=np.float32)
    sw2 = np.asarray(inputs["sw2"], dtype=np.float32)
    sb2 = np.asarray(inputs["sb2"], dtype=np.float32)
    mw1 = np.asarray(inputs["mw1"], dtype=np.float32)
    mb1 = np.asarray(inputs["mb1"], dtype=np.float32)
    mw2 = np.asarray(inputs["mw2"], dtype=np.float32)
    mb2 = np.asarray(inputs["mb2"], dtype=np.float32)
    mw3 = np.asarray(inputs["mw3"], dtype=np.float32)
    mb3 = np.asarray(inputs["mb3"], dtype=np.float32)

    import ml_dtypes
    f8np = ml_dtypes.float8_e4m3

    # flat blocked layout: per (slide, partition) the macros are packed
    # back-to-back, each one a contiguous KC*nq run (clean DMA descriptors
    # for every macro width); the first slide of each core ramps finer
    xtf = f[:, :, META:].transpose(0, 2, 1).astype(f8np)        # (B, D, N)
    xr = xtf.reshape(B, KC, 128, N)
    xm = np.zeros((B, 128, KC * N), f8np)
    for g in range(B):
        tab = MACROS0 if (g % BPC == 0) else MACROSR
        n0 = 0
        for nq in tab:
            blk = xr[g, :, :, n0 : n0 + nq].transpose(1, 0, 2)  # (128, KC, nq)
            xm[g, :, KC * n0 : KC * (n0 + nq)] = blk.reshape(128, KC * nq)
            n0 += nq
    mb1p = (mb1 + sb2[0] * mw1.sum(axis=1)).astype(np.float32)
    # bottom-100 ext values arrive sign-flipped (max8 of -s); negate the
    # corresponding m1 columns instead of negating on-device
    mw1k = mw1.copy()
    mw1k[:, NTOP:] *= -1.0

    # w1 blocked [p, k*128+h] = w1[d=128k+p, h] (one descriptor per partition)
    w1blk = np.ascontiguousarray(
        (sw1.T * WSCALE).reshape(KC, 128, 128).transpose(1, 0, 2).reshape(128, KC * 128)
    ).astype(f8np)

    m1tk = mw1k.T.astype(np.float16)            # (200, 128)
    wblob = np.zeros((128, 322), np.float16)
    wblob[:, 0:128] = m1tk[0:128, :]
    wblob[:, 128:192] = mw2.T.astype(np.float16)
    wblob[0:72, 192:320] = m1tk[128:200, :]
    wblob[0:64, 320] = mw3.T[:, 0].astype(np.float16)
    wblob[:, 321] = sw2[0, :].astype(np.float16)
    brows = np.zeros((4, 128), np.float32)
    brows[0, :] = sb1
    brows[1, :] = mb1p
    brows[2, 0:64] = mb2
    brows[3, 0] = mb3[0]

    common = {"w1t": w1blk, "wblob": wblob, "brows": brows}
    in_maps = [
        {"xt": xm[c * BPC : (c + 1) * BPC], **common}
        for c in range(NCORES)
    ]

    res = run_bass_kernel_spmd(nc, in_maps, core_ids=list(range(NCORES)))
    LAST_RESULT = res
    out = np.concatenate([r["y"].reshape(BPC) for r in res.results])
    return out.reshape(B, 1).astype(np.float32)

